# revision 1
# baseline (speedup 1.0000x reference)
"""Trainium2 Bass kernel for FovConv2dCont (per-pixel foveated Gaussian blur + 5x5 conv).

kernel(**inputs): takes FULL inputs
  input_data f32 (8,3,224,224), foa_xy int (8,2), weight f32 (64,3,5,5)
returns f32 (8,64,224,224). Batch is data-parallel across 8 NeuronCores (1 sample/core).

Math (exact identities; bf16 storage on the heavy elementwise chain):
  gaussian tap exp(-(i^2+j^2)/(2 s^2)) = u^(i^2) * u^(j^2),  u = exp(-1/(2 s^2))
  normalizer sum over 7x7 taps = (1 + 2u + 2u^4 + 2u^9)^2
  numerator = sum_{a,b in 0..3} u^(a^2+b^2) P_ab,  P_ab = rowpair_a(colpair_b(x))
  (terms e=13,18 dropped: bounded by ~2e-3 relative, below bf16 noise)
  m = numerator / s^2 ; y = conv5x5(m, w) as K=120 matmuls with (ci,dy,dx) on the
  partition axis of an im2col-lite buffer; 4 weight variants pre-shifted by output
  row mod 4 so the matmul partition window is always [0,120).
"""

import os
import sys

sys.path.insert(0, "/opt/trn_rl_repo")

import numpy as np
import ml_dtypes

def _ensure_ntff_hook():
    """Register the NTFF profile hook if the image's antenv lacks axon_hooks
    (needed only for trace=True timing runs; harmless otherwise)."""
    try:
        import antenv.axon_hooks  # noqa: F401
        return
    except ImportError:
        pass
    try:
        import types
        import antenv
        import importlib.util as ilu

        spec = ilu.spec_from_file_location(
            "trn_agent_boot.trn_boot", "/root/.axon_site/trn_agent_boot/trn_boot.py"
        )
        mod = types.ModuleType("antenv.axon_hooks")
        _hook_holder = {"hook": None}

        def set_axon_ntff_profile_hook(h):
            _hook_holder["hook"] = h

        def get_axon_ntff_profile_hook():
            return _hook_holder["hook"]

        mod.set_axon_ntff_profile_hook = set_axon_ntff_profile_hook
        mod.get_axon_ntff_profile_hook = get_axon_ntff_profile_hook
        sys.modules["antenv.axon_hooks"] = mod
        antenv.axon_hooks = mod

        boot = ilu.module_from_spec(spec)
        spec.loader.exec_module(boot)
        hook = boot._ntff_profile_via_ctypes("/opt/axon/libaxon_pjrt.so")
        set_axon_ntff_profile_hook(hook)
    except Exception:
        pass


_ensure_ntff_hook()

import concourse.bass as bass
import concourse.bacc as bacc_mod
import concourse.mybir as mybir
from concourse.bass_utils import run_bass_kernel_spmd
from concourse.tile import TileContext
from concourse.alu_op_type import AluOpType

F32 = mybir.dt.float32
BF16 = mybir.dt.bfloat16
AF = mybir.ActivationFunctionType

H = W = 224
C = 3
OC = 64
KG = 7
PG = KG // 2            # 3
KC = 5
PC = KC // 2            # 2
WP = W + 2 * PG         # 230
SR = 8                  # strip rows per partition
NP = H // 2             # 112 partitions
MW = W + 2 * PC         # 228
DNORM = float(np.sqrt(H * H + W * W))
NBLK = H // 4           # 56 row blocks

STRIPES = [56, 56, 56, 56]

EXPS_PAIR = {1: (0, 1), 4: (0, 2), 5: (1, 2), 9: (0, 3), 10: (1, 3)}
EXPS_DIAG = {2: 1, 8: 2}
ALL_E = sorted(set(EXPS_PAIR) | set(EXPS_DIAG))

LAST_RESULTS = None
_CACHED = None


def _v(ap_src, offset_elems, dims):
    """Raw strided (possibly overlapping/broadcast) view of a flat AP.
    dims = [(step, count), ...]; for SBUF/PSUM the first dim(s) must cover
    partitions (step in flat units = partition_step * free_size)."""
    fv = ap_src.flatten()
    v = fv.copy()
    v.offset = fv.offset + offset_elems
    v.ap = mybir.VecI64Pair([list(d) for d in dims])
    return v


def _build_nc():
    nc = bacc_mod.Bacc()

    xp = nc.declare_dram_parameter("xp", [C, WP, WP], BF16, isOutput=False)
    av = nc.declare_dram_parameter("av", [H], F32, isOutput=False)
    bv = nc.declare_dram_parameter("bv", [H], F32, isOutput=False)
    wb = nc.declare_dram_parameter("wb", [4, 120, OC], BF16, isOutput=False)
    zv = nc.declare_dram_parameter("zv", [2 * C * MW], BF16, isOutput=False)
    out = nc.declare_dram_parameter("out", [OC, H, W], F32, isOutput=True)

    with TileContext(nc) as tc:
        with (
            tc.tile_pool(name="pers", bufs=1) as pers,
            tc.tile_pool(name="psum", bufs=8, space="PSUM") as psum_pool,
            tc.tile_pool(name="stage", bufs=4) as stage_pool,
            tc.tile_pool(name="dram", bufs=1, space="DRAM") as dram_pool,
        ):
            mpad = dram_pool.tile([C, MW, MW], BF16)
            XFS = C * SR * WP                       # xs free size 5520
            xs = pers.tile([NP, XFS], BF16)
            CFS = 2 * W                             # coeff free size 448
            at = pers.tile([NP, 2], F32)
            bvf = pers.tile([NP, W], F32)
            d2 = pers.tile([NP, CFS], F32)
            dist = pers.tile([NP, CFS], F32)
            sig = pers.tile([NP, CFS], F32)
            sqv = pers.tile([NP, CFS], F32)
            isg = pers.tile([NP, CFS], F32)
            u1f = pers.tile([NP, CFS], F32)
            u4f = pers.tile([NP, CFS], F32)
            u9f = pers.tile([NP, CFS], F32)
            t1 = pers.tile([NP, CFS], F32)
            t2 = pers.tile([NP, CFS], F32)
            sfield = pers.tile([NP, CFS], F32)
            rsf = pers.tile([NP, CFS], F32)
            rb = pers.tile([NP, CFS], BF16)
            ub = {e: pers.tile([NP, CFS], BF16, name=f"ub{e}") for e in ALL_E}
            RFS = C * 2 * WP                        # rowpair free size 1380
            rp = {a: pers.tile([NP, RFS], BF16, name=f"rp{a}") for a in (1, 2, 3)}
            PFS = C * 2 * W                         # P tile free size 1344
            ptiles = {}
            for e, (a, b) in EXPS_PAIR.items():
                ptiles[(a, b)] = pers.tile([NP, PFS], BF16, name=f"p{a}{b}")
                if a != 0:
                    ptiles[(b, a)] = pers.tile([NP, PFS], BF16, name=f"p{b}{a}")
            for e, a in EXPS_DIAG.items():
                ptiles[(a, a)] = pers.tile([NP, PFS], BF16, name=f"pd{a}")
            qtiles = {e: pers.tile([NP, PFS], BF16, name=f"q{e}") for e in EXPS_PAIR}
            prod = pers.tile([NP, PFS], BF16)
            acc = pers.tile([NP, PFS], BF16)
            acc2 = pers.tile([NP, PFS], BF16)
            MFS = C * 2 * MW                        # m free size 1368
            mt = pers.tile([NP, MFS], BF16)
            wtile = pers.tile([120, 4 * OC], BF16)
            imt = {
                si: pers.tile([120, NBLK * SW], BF16, name=f"imt{si}")
                for si, SW in enumerate(STRIPES)
            }

            # ---------------- loads + zero fills ----------------
            nc.vector.memset(mt[:], 0.0)

            for ci in range(C):
                src = _v(xp[ci], 0, [[2 * WP, NP], [WP, SR], [1, WP]])
                dst = _v(xs[:], ci * SR * WP, [[XFS, NP], [WP, SR], [1, WP]])
                nc.sync.dma_start(out=dst, in_=src)

            for base in (0, MW - 2):
                dst = _v(mpad[:], base * MW, [[MW * MW, C], [MW, 2], [1, MW]])
                src = _v(zv[:], 0, [[2 * MW, C], [MW, 2], [1, MW]])
                nc.sync.dma_start(out=dst, in_=src)

            nc.sync.dma_start(
                out=_v(at[:], 0, [[2, NP], [1, 2]]),
                in_=_v(av[:], 0, [[2, NP], [1, 2]]),
            )
            nc.sync.dma_start(
                out=_v(bvf[:], 0, [[W, NP], [1, W]]),
                in_=_v(bv[:], 0, [[0, NP], [1, W]]),
            )
            nc.sync.dma_start(
                out=_v(wtile[:], 0, [[4 * OC, 120], [OC, 4], [1, OC]]),
                in_=_v(wb[:], 0, [[OC, 120], [120 * OC, 4], [1, OC]]),
            )

            # ---------------- shared coefficient chain ----------------
            for rh in range(2):
                nc.vector.tensor_scalar(
                    d2[:, rh * W:(rh + 1) * W], bvf[:],
                    at[:, rh:rh + 1], None, AluOpType.add,
                )
            nc.scalar.activation(dist[:], d2[:], AF.Sqrt)
            nc.scalar.activation(sig[:], dist[:], AF.Copy, bias=0.01, scale=0.99)
            nc.scalar.activation(sqv[:], sig[:], AF.Square)
            nc.vector.reciprocal(isg[:], sqv[:])
            nc.scalar.activation(u1f[:], isg[:], AF.Exp, scale=-0.5)
            nc.scalar.activation(u4f[:], isg[:], AF.Exp, scale=-2.0)
            nc.scalar.activation(u9f[:], isg[:], AF.Exp, scale=-4.5)
            for e in ALL_E:
                nc.scalar.activation(ub[e][:], isg[:], AF.Exp, scale=-0.5 * e)
            nc.vector.tensor_tensor(t1[:], u1f[:], u4f[:], AluOpType.add)
            nc.vector.tensor_tensor(t2[:], t1[:], u9f[:], AluOpType.add)
            nc.vector.tensor_scalar(
                sfield[:], t2[:], 2.0, 1.0, AluOpType.mult, AluOpType.add
            )
            nc.vector.reciprocal(rsf[:], sfield[:])
            nc.scalar.activation(rb[:], rsf[:], AF.Square)

            # rowpairs, full padded width
            for a in (1, 2, 3):
                i0 = _v(xs[:], (PG - a) * WP,
                        [[XFS, NP], [SR * WP, C], [WP, 2], [1, WP]])
                i1 = _v(xs[:], (PG + a) * WP,
                        [[XFS, NP], [SR * WP, C], [WP, 2], [1, WP]])
                o = _v(rp[a][:], 0, [[RFS, NP], [2 * WP, C], [WP, 2], [1, WP]])
                nc.vector.tensor_tensor(o, i0, i1, AluOpType.add)

            # ---------------- per-stripe pipeline ----------------
            qs = 0
            for si, SW in enumerate(STRIPES):
                mlo = max(qs - PC, 0)
                mhi = min(qs + SW + PC, W)
                MWD = mhi - mlo

                def xsv(col_off, _m=mlo, _w=MWD):
                    return _v(xs[:], PG * WP + PG + _m + col_off,
                              [[XFS, NP], [SR * WP, C], [WP, 2], [1, _w]])

                def rpv(a, col_off, _m=mlo, _w=MWD):
                    return _v(rp[a][:], PG + _m + col_off,
                              [[RFS, NP], [2 * WP, C], [WP, 2], [1, _w]])

                def pv(t, _m=mlo, _w=MWD):
                    return _v(t[:], _m, [[PFS, NP], [2 * W, C], [W, 2], [1, _w]])

                def uv(t, _m=mlo, _w=MWD):
                    return _v(t[:], _m, [[CFS, NP], [0, C], [W, 2], [1, _w]])

                # colpairs
                for (a, b), pt in ptiles.items():
                    if b == 0:
                        continue
                    if a == 0:
                        i0, i1 = xsv(-b), xsv(+b)
                    else:
                        i0, i1 = rpv(a, -b), rpv(a, +b)
                    nc.vector.tensor_tensor(pv(pt), i0, i1, AluOpType.add)

                # Q pre-adds
                for e, (a, b) in EXPS_PAIR.items():
                    second = rpv(b, 0) if a == 0 else pv(ptiles[(b, a)])
                    nc.vector.tensor_tensor(
                        pv(qtiles[e]), pv(ptiles[(a, b)]), second, AluOpType.add
                    )

                # products + accumulation
                terms = [
                    (e, qtiles[e] if e in EXPS_PAIR else ptiles[(EXPS_DIAG[e],) * 2])
                    for e in ALL_E
                ]
                accs = [acc, acc2]
                cur = None
                for ti, (e, qt) in enumerate(terms):
                    nc.vector.tensor_tensor(pv(prod), uv(ub[e]), pv(qt),
                                            AluOpType.mult)
                    nxt = accs[ti % 2]
                    first = xsv(0) if ti == 0 else pv(cur)
                    nc.vector.tensor_tensor(pv(nxt), first, pv(prod), AluOpType.add)
                    cur = nxt

                # m = rb * acc
                mdst = _v(mt[:], PC + mlo,
                          [[MFS, NP], [2 * MW, C], [MW, 2], [1, MWD]])
                nc.vector.tensor_tensor(mdst, uv(rb), pv(cur), AluOpType.mult)

                # m -> mpad DRAM (padded cols [qs, qs+SW+4))
                plo, phi = qs, qs + SW + 2 * PC
                PWD = phi - plo
                for ci in range(C):
                    src = _v(mt[:], ci * 2 * MW + plo,
                             [[MFS, NP], [MW, 2], [1, PWD]])
                    dst = _v(mpad[:], ci * MW * MW + 2 * MW + plo,
                             [[2 * MW, NP], [MW, 2], [1, PWD]])
                    nc.sync.dma_start(out=dst, in_=src)

                # im2col: imt[(ci*40+dx*8+dr), (blk, c)] = mpad[ci, 4blk+dr, qs+dx+c]
                it = imt[si]
                IFS = NBLK * SW
                for ci in range(C):
                    for dx in range(KC):
                        src = _v(mpad[:], ci * MW * MW + qs + dx,
                                 [[MW, SR], [4 * MW, NBLK], [1, SW]])
                        dst = _v(it[:], (ci * 40 + dx * 8) * IFS,
                                 [[IFS, SR], [SW, NBLK], [1, SW]])
                        nc.scalar.dma_start(out=dst, in_=src)

                # matmuls + output: 8 row-pairs (16 rows) per PSUM bank,
                # PSUM -> SBUF via one wide ScalarE copy, then SBUF -> DRAM.
                NPAIR = 8
                for g in range(H // (2 * NPAIR)):           # 14 groups
                    pt = psum_pool.tile([128, NPAIR * SW], F32, name="opsum")
                    for pair in range(NPAIR):
                        qr0 = g * 2 * NPAIR + 2 * pair
                        blk = qr0 // 4
                        for parity in range(2):
                            qr = qr0 + parity
                            rhs = _v(it[:], blk * SW, [[IFS, 120], [1, SW]])
                            lhsT = _v(wtile[:], (qr % 4) * OC,
                                      [[4 * OC, 120], [1, OC]])
                            nc.tensor.matmul(
                                pt[parity * OC:(parity + 1) * OC,
                                   pair * SW:(pair + 1) * SW],
                                lhsT, rhs,
                                start=True, stop=True,
                                tile_position=(0, parity * OC),
                            )
                    st = stage_pool.tile([128, NPAIR * SW], F32, name="ostage")
                    nc.scalar.copy(st[:], pt[:])
                    for pair in range(NPAIR):
                        qr0 = g * 2 * NPAIR + 2 * pair
                        for parity in range(2):
                            dst = _v(out[:], (qr0 + parity) * W + qs,
                                     [[H * W, OC], [1, SW]])
                            src = _v(st[:],
                                     parity * OC * NPAIR * SW + pair * SW,
                                     [[NPAIR * SW, OC], [1, SW]])
                            nc.sync.dma_start(out=dst, in_=src)

                qs += SW

    return nc


def _get_nc():
    global _CACHED
    if _CACHED is None:
        nc = _build_nc()
        nc.finalize()
        _CACHED = nc
    return _CACHED


def _host_prep(input_data, foa_xy, weight):
    b = input_data.shape[0]
    wbs = np.zeros((4, 120, OC), dtype=np.float32)
    for v in range(4):
        for ci in range(C):
            for dy in range(KC):
                for dx in range(KC):
                    k = ci * 40 + dx * 8 + dy + v
                    wbs[v, k, :] = weight[:, ci, dy, dx]
    wbs = wbs.astype(ml_dtypes.bfloat16)
    idx = np.arange(H, dtype=np.float64)
    in_maps = []
    for i in range(b):
        xpad = np.zeros((C, WP, WP), dtype=ml_dtypes.bfloat16)
        xpad[:, PG:PG + H, PG:PG + W] = input_data[i].astype(ml_dtypes.bfloat16)
        fx, fy = float(foa_xy[i, 0]), float(foa_xy[i, 1])
        a_sq = (((idx - fx) / DNORM) ** 2).astype(np.float32)
        b_sq = (((idx - fy) / DNORM) ** 2).astype(np.float32)
        zvz = np.zeros(2 * C * MW, dtype=ml_dtypes.bfloat16)
        in_maps.append({"xp": xpad, "av": a_sq, "bv": b_sq, "wb": wbs, "zv": zvz})
    return in_maps


def kernel(input_data, foa_xy, weight):
    global LAST_RESULTS
    nc = _get_nc()
    in_maps = _host_prep(np.asarray(input_data), np.asarray(foa_xy),
                         np.asarray(weight))
    trace = bool(int(os.environ.get("BASSKERNEL_TRACE", "0")))
    res = run_bass_kernel_spmd(nc, in_maps, core_ids=list(range(8)), trace=trace)
    LAST_RESULTS = res
    outs = [np.asarray(r["out"], dtype=np.float32) for r in res.results]
    return np.stack(outs, axis=0)



# revision 10
# speedup vs baseline: 4.2847x; 4.2847x over previous
"""Trainium2 Bass kernel for FovConv2dCont (per-pixel foveated Gaussian blur + 5x5 conv).

kernel(**inputs): takes FULL inputs
  input_data f32 (8,3,224,224), foa_xy int (8,2), weight f32 (64,3,5,5)
returns f32 (8,64,224,224). Batch is data-parallel across 8 NeuronCores (1 sample/core).

Math (exact identities; bf16 storage on the heavy elementwise chain):
  gaussian tap exp(-(i^2+j^2)/(2 s^2)) = u^(i^2) * u^(j^2),  u = exp(-1/(2 s^2))
  normalizer sum over 7x7 taps = (1 + 2u + 2u^4 + 2u^9)^2
  numerator = sum over exponent classes e in {0,1,2,4,5,8,9} of u^e * S_e
  (terms e=10,13,18 dropped: bounded ~5e-3 relative, within tolerance)
  m = numerator / norm ; y = conv5x5(m, w) as K=120 matmuls with (ci,dx,dr) on the
  partition axis of an im2col buffer; 4 weight variants pre-shifted by output
  row mod 4 so the matmul partition window is always [0,120).

Pipeline per core: full-width Gaussian chain split across DVE+Pool engines ->
m field -> DRAM (mpad) -> 4 im2col DMAs -> 56 matmuls [120,128]x[120,448] ->
PSUM->SBUF copies (Scalar+Pool) -> 28 wide output DMAs (16 rows each).
"""

import os
import sys

sys.path.insert(0, "/opt/trn_rl_repo")

import numpy as np
import ml_dtypes

def _ensure_ntff_hook():
    """Register the NTFF profile hook if the image's antenv lacks axon_hooks
    (needed only for trace=True timing runs; harmless otherwise)."""
    try:
        import antenv.axon_hooks  # noqa: F401
        return
    except ImportError:
        pass
    try:
        import types
        import antenv
        import importlib.util as ilu

        spec = ilu.spec_from_file_location(
            "trn_agent_boot.trn_boot", "/root/.axon_site/trn_agent_boot/trn_boot.py"
        )
        mod = types.ModuleType("antenv.axon_hooks")
        _hook_holder = {"hook": None}

        def set_axon_ntff_profile_hook(h):
            _hook_holder["hook"] = h

        def get_axon_ntff_profile_hook():
            return _hook_holder["hook"]

        mod.set_axon_ntff_profile_hook = set_axon_ntff_profile_hook
        mod.get_axon_ntff_profile_hook = get_axon_ntff_profile_hook
        sys.modules["antenv.axon_hooks"] = mod
        antenv.axon_hooks = mod

        boot = ilu.module_from_spec(spec)
        spec.loader.exec_module(boot)
        hook = boot._ntff_profile_via_ctypes("/opt/axon/libaxon_pjrt.so")
        set_axon_ntff_profile_hook(hook)
    except Exception:
        pass


_ensure_ntff_hook()

import concourse.bass as bass
import concourse.bacc as bacc_mod
import concourse.mybir as mybir
from concourse.bass_utils import run_bass_kernel_spmd
from concourse.tile import TileContext
from concourse.alu_op_type import AluOpType

F32 = mybir.dt.float32
BF16 = mybir.dt.bfloat16
AF = mybir.ActivationFunctionType

H = W = 224
C = 3
OC = 64
KG = 7
PG = KG // 2            # 3
KC = 5
PC = KC // 2            # 2
XW = 256                # padded input row width (512B bf16 rows)
XR = H + 2 * PG         # 230 padded input rows
SR = 8                  # input rows per partition (2 out rows + gaussian halo)
NP = H // 2             # 112 partitions (row pairs)
MW = 260                # mpad row width: m cols at [2,226), im2col reads [dx, dx+256)
MR = H + 2 * PC         # 228 mpad rows
DNORM = float(np.sqrt(H * H + W * W))
NBLK = H // 4           # 56 conv row blocks
NG = 4                  # im2col block groups
GB = NBLK // NG         # 14 blocks per group
IFS = NBLK * XW         # imt free size = 56*256 = 14336
OG = 14                 # output groups (16 rows each)

# exponent classes kept (e = a^2 + b^2 over taps 0..3); 10,13,18 dropped
E_PAIR = {1: (0, 1), 4: (0, 2), 5: (1, 2), 9: (0, 3)}
E_DIAG = {2: 1, 8: 2}

LAST_RESULTS = None
_CACHED = None


def _v(ap_src, offset_elems, dims):
    """Raw strided (possibly overlapping/broadcast) view of a flat AP.
    dims = [(step, count), ...]; for SBUF/PSUM the first dim(s) must cover
    partitions (step in flat units = partition_step * free_size)."""
    fv = ap_src.flatten()
    v = fv.copy()
    v.offset = fv.offset + offset_elems
    v.ap = mybir.VecI64Pair([list(d) for d in dims])
    return v


def _build_nc():
    nc = bacc_mod.Bacc()

    xp = nc.declare_dram_parameter("xp", [C, XR, XW], BF16, isOutput=False)
    av = nc.declare_dram_parameter("av", [H], F32, isOutput=False)
    bv = nc.declare_dram_parameter("bv", [H], F32, isOutput=False)
    wb = nc.declare_dram_parameter("wb", [4, 120, OC], BF16, isOutput=False)
    out = nc.declare_dram_parameter("out", [OC, H, W], F32, isOutput=True)

    with TileContext(nc) as tc:
        with (
            tc.tile_pool(name="pers", bufs=1) as pers,
            tc.tile_pool(name="psum", bufs=8, space="PSUM") as psum_pool,
            tc.tile_pool(name="stage", bufs=3) as stage_pool,
            tc.tile_pool(name="dram", bufs=1, space="DRAM") as dram_pool,
        ):
            mpad = dram_pool.tile([C, MR, MW], BF16)

            XFS = C * SR * XW                       # xs free size 6144
            xs = pers.tile([NP, XFS], BF16)
            CFS = 2 * W                             # coeff free size 448
            at = pers.tile([NP, 2], F32)
            bvf = pers.tile([NP, W], F32)
            d2 = pers.tile([NP, CFS], F32)
            dist = pers.tile([NP, CFS], F32)
            sig = pers.tile([NP, CFS], F32)
            sqv = pers.tile([NP, CFS], F32)
            isg = pers.tile([NP, CFS], F32)
            u1f = pers.tile([NP, CFS], F32)
            u4f = pers.tile([NP, CFS], F32)
            u9f = pers.tile([NP, CFS], F32)
            t1 = pers.tile([NP, CFS], F32)
            t2 = pers.tile([NP, CFS], F32)
            sfield = pers.tile([NP, CFS], F32)
            rsf = pers.tile([NP, CFS], F32)
            rb = pers.tile([NP, CFS], BF16)
            ub = {e: pers.tile([NP, CFS], BF16, name=f"ub{e}")
                  for e in (1, 2, 4, 5, 8, 9)}
            RFS = C * 2 * XW                        # rowpair free size 1536
            rp = {a: pers.tile([NP, RFS], BF16, name=f"rp{a}") for a in (1, 2, 3)}
            PFS = C * 2 * W                         # P/S/T free size 1344
            pt = {k: pers.tile([NP, PFS], BF16, name=f"p{k[0]}{k[1]}")
                  for k in ((0, 1), (0, 2), (0, 3), (1, 1), (1, 2), (2, 1), (2, 2))}
            st = {e: pers.tile([NP, PFS], BF16, name=f"s{e}") for e in E_PAIR}
            tt = {e: pers.tile([NP, PFS], BF16, name=f"t{e}")
                  for e in (1, 2, 4, 5, 8, 9)}
            aa = [pers.tile([NP, PFS], BF16, name=f"aa{i}") for i in range(3)]
            ab = [pers.tile([NP, PFS], BF16, name=f"ab{i}") for i in range(2)]
            accf = pers.tile([NP, PFS], BF16)
            MFS = C * 2 * MW                        # mt free size 1560
            mt = pers.tile([NP, MFS], BF16)
            zt = pers.tile([C, 2 * 2 * MW], BF16)   # zero rows for mpad borders
            wtile = pers.tile([120, 4 * OC], BF16)
            imt = pers.tile([120, IFS], BF16)

            # ---------------- memsets (Pool engine, early) ----------------
            nc.gpsimd.memset(zt[:], 0.0)
            nc.gpsimd.memset(mt[:], 0.0)

            # ---------------- loads (sync engine) ----------------
            for ci in range(C):
                src = _v(xp[:], ci * XR * XW, [[2 * XW, NP], [XW, SR], [1, XW]])
                dst = _v(xs[:], ci * SR * XW, [[XFS, NP], [XW, SR], [1, XW]])
                nc.sync.dma_start(out=dst, in_=src)
            nc.sync.dma_start(
                out=_v(at[:], 0, [[2, NP], [1, 2]]),
                in_=_v(av[:], 0, [[2, NP], [1, 2]]),
            )
            nc.sync.dma_start(
                out=_v(bvf[:], 0, [[W, NP], [1, W]]),
                in_=_v(bv[:], 0, [[0, NP], [1, W]]),
            )
            nc.sync.dma_start(
                out=_v(wtile[:], 0, [[4 * OC, 120], [OC, 4], [1, OC]]),
                in_=_v(wb[:], 0, [[OC, 120], [120 * OC, 4], [1, OC]]),
            )
            # mpad top/bottom zero rows (rows 0-1 and 226-227, all ci)
            nc.sync.dma_start(
                out=_v(mpad[:], 0, [[MR * MW, C], [(MR - 2) * MW, 2], [1, 2 * MW]]),
                in_=_v(zt[:], 0, [[4 * MW, C], [2 * MW, 2], [1, 2 * MW]]),
            )

            # ---------------- coefficient chain ----------------
            for rh in range(2):
                nc.vector.tensor_scalar(
                    d2[:, rh * W:(rh + 1) * W], bvf[:],
                    at[:, rh:rh + 1], None, AluOpType.add,
                )
            nc.scalar.activation(dist[:], d2[:], AF.Sqrt)
            nc.scalar.activation(sig[:], dist[:], AF.Copy, bias=0.01, scale=0.99)
            nc.scalar.activation(sqv[:], sig[:], AF.Square)
            nc.vector.reciprocal_approx_fast(isg[:], sqv[:])
            nc.scalar.activation(u1f[:], isg[:], AF.Exp, scale=-0.5)
            nc.scalar.activation(u4f[:], isg[:], AF.Exp, scale=-2.0)
            nc.scalar.activation(u9f[:], isg[:], AF.Exp, scale=-4.5)
            for e in (2, 5, 8):
                nc.scalar.activation(ub[e][:], isg[:], AF.Exp, scale=-0.5 * e)
            nc.vector.tensor_copy(ub[1][:], u1f[:])
            nc.vector.tensor_copy(ub[4][:], u4f[:])
            nc.vector.tensor_copy(ub[9][:], u9f[:])
            nc.vector.tensor_tensor(t1[:], u1f[:], u4f[:], AluOpType.add)
            nc.vector.tensor_tensor(t2[:], t1[:], u9f[:], AluOpType.add)
            nc.vector.tensor_scalar(
                sfield[:], t2[:], 2.0, 1.0, AluOpType.mult, AluOpType.add
            )
            nc.vector.reciprocal_approx_fast(rsf[:], sfield[:])
            nc.scalar.activation(rb[:], rsf[:], AF.Square)

            # ---------------- gaussian chain views ----------------
            # xs free layout: (ci, row 0..7, col 0..255); out row r of the pair
            # sits at in-partition row 3+r, center col c at 3+c.
            def xv(drow, dcol, wid):
                return _v(xs[:], (3 + drow) * XW + 3 + dcol,
                          [[XFS, NP], [SR * XW, C], [XW, 2], [1, wid]])

            def rv(t, dcol, wid):
                return _v(t[:], 3 + dcol,
                          [[RFS, NP], [2 * XW, C], [XW, 2], [1, wid]])

            def pv(t):
                return _v(t[:], 0, [[PFS, NP], [2 * W, C], [W, 2], [1, W]])

            def uv(t):
                return _v(t[:], 0, [[CFS, NP], [0, C], [W, 2], [1, W]])

            # rowpairs on DVE (full padded width)
            for a in (1, 2, 3):
                o = _v(rp[a][:], 0, [[RFS, NP], [2 * XW, C], [XW, 2], [1, XW]])
                i0 = _v(xs[:], (3 - a) * XW,
                        [[XFS, NP], [SR * XW, C], [XW, 2], [1, XW]])
                i1 = _v(xs[:], (3 + a) * XW,
                        [[XFS, NP], [SR * XW, C], [XW, 2], [1, XW]])
                nc.vector.tensor_tensor(o, i0, i1, AluOpType.add)

            # colpairs: P[a][b] = (a-rowpair or x) shifted by +-b, summed.
            # DVE: P11 P12 ; Pool: P01 P02 P03 P21 P22
            for (a, b), eng in (((1, 1), nc.vector), ((1, 2), nc.vector),
                                ((2, 1), nc.gpsimd), ((2, 2), nc.gpsimd),
                                ((0, 1), nc.gpsimd), ((0, 2), nc.gpsimd),
                                ((0, 3), nc.gpsimd)):
                if a == 0:
                    i0, i1 = xv(0, -b, W), xv(0, +b, W)
                else:
                    i0, i1 = rv(rp[a], -b, W), rv(rp[a], +b, W)
                eng.tensor_tensor(pv(pt[(a, b)]), i0, i1, AluOpType.add)

            # class sums S_e: S1=P01+rp1, S4=P02+rp2, S5=P12+P21, S9=P03+rp3
            nc.vector.tensor_tensor(pv(st[5]), pv(pt[(1, 2)]), pv(pt[(2, 1)]),
                                    AluOpType.add)
            nc.gpsimd.tensor_tensor(pv(st[1]), pv(pt[(0, 1)]), rv(rp[1], 0, W),
                                    AluOpType.add)
            nc.gpsimd.tensor_tensor(pv(st[4]), pv(pt[(0, 2)]), rv(rp[2], 0, W),
                                    AluOpType.add)
            nc.gpsimd.tensor_tensor(pv(st[9]), pv(pt[(0, 3)]), rv(rp[3], 0, W),
                                    AluOpType.add)

            # products T_e = ub_e * S_e   (S2=P11, S8=P22)
            src_e = {1: st[1], 4: st[4], 5: st[5], 9: st[9],
                     2: pt[(1, 1)], 8: pt[(2, 2)]}
            for e, eng in ((2, nc.vector), (5, nc.vector), (8, nc.gpsimd),
                           (1, nc.gpsimd), (4, nc.gpsimd), (9, nc.gpsimd)):
                eng.tensor_tensor(pv(tt[e]), uv(ub[e]), pv(src_e[e]),
                                  AluOpType.mult)

            # accumulation tree: DVE: ab0=x+T2, ab1=ab0+T5
            #                    Pool: aa0=T1+T8, aa1=aa0+T4, aa2=aa1+T9
            nc.vector.tensor_tensor(pv(ab[0]), xv(0, 0, W), pv(tt[2]), AluOpType.add)
            nc.vector.tensor_tensor(pv(ab[1]), pv(ab[0]), pv(tt[5]), AluOpType.add)
            nc.gpsimd.tensor_tensor(pv(aa[0]), pv(tt[1]), pv(tt[8]), AluOpType.add)
            nc.gpsimd.tensor_tensor(pv(aa[1]), pv(aa[0]), pv(tt[4]), AluOpType.add)
            nc.gpsimd.tensor_tensor(pv(aa[2]), pv(aa[1]), pv(tt[9]), AluOpType.add)
            nc.vector.tensor_tensor(pv(accf), pv(aa[2]), pv(ab[1]), AluOpType.add)

            # m = rb * acc, into mt center cols [2, 226)
            mdst = _v(mt[:], 2, [[MFS, NP], [2 * MW, C], [MW, 2], [1, W]])
            nc.vector.tensor_tensor(mdst, uv(rb), pv(accf), AluOpType.mult)

            # m rows -> mpad (full 260-wide rows incl zero borders)
            for ci in range(C):
                src = _v(mt[:], ci * 2 * MW, [[MFS, NP], [1, 2 * MW]])
                dst = _v(mpad[:], ci * MR * MW + 2 * MW,
                         [[2 * MW, NP], [1, 2 * MW]])
                nc.sync.dma_start(out=dst, in_=src)

            # im2col: imt[(ci*40+dx*8+dr), blk*256+j] = mpad[ci, 4blk+dr, dx+j]
            # (DMA APs are limited to 3 dims per side -> one DMA per (ci, dx))
            for ci in range(C):
                for dx in range(KC):
                    src = _v(mpad[:], ci * MR * MW + dx,
                             [[MW, SR], [4 * MW, NBLK], [1, XW]])
                    dst = _v(imt[:], (ci * 40 + dx * 8) * IFS,
                             [[IFS, SR], [XW, NBLK], [1, XW]])
                    eng = nc.sync if (ci * KC + dx) % 2 == 0 else nc.scalar
                    eng.dma_start(out=dst, in_=src)

            # ---------------- conv: matmuls + copies + output DMA ----------------
            # group og covers out rows [32*og, 32*og+32) = blocks 8*og..8*og+7.
            # matmul (og, vp, h): lhsT = wtile[:, vp*128:(vp+1)*128] (variants
            # 2vp, 2vp+1), rhs = imt blocks (8og+2h, 8og+2h+1) -> psum [128,448];
            # psum partition p = vhalf*64+oc -> out row 4*blk+2vp+vhalf.
            SGB = 8                                 # blocks per output group
            for og in range(NBLK // SGB):
                stg = stage_pool.tile([128, SGB * CFS], F32, name="ostage")
                for vp in range(2):
                    lhsT = _v(wtile[:], vp * 128, [[4 * OC, 120], [1, 128]])
                    for h in range(SGB // 2):
                        ps = psum_pool.tile([128, CFS], F32, name="opsum")
                        rhs = _v(imt[:], (SGB * og + 2 * h) * XW,
                                 [[IFS, 120], [XW, 2], [1, W]])
                        nc.tensor.matmul(ps[:], lhsT, rhs, start=True, stop=True)
                        cdst = _v(stg[:], h * 2 * CFS + vp * W,
                                  [[SGB * CFS, 128], [CFS, 2], [1, W]])
                        csrc = _v(ps[:], 0, [[CFS, 128], [W, 2], [1, W]])
                        if vp == 0:
                            nc.scalar.copy(cdst, csrc)
                        else:
                            nc.vector.tensor_copy(cdst, csrc)
                # stage free layout is (b, vp, c) = (2b+vp)*224 + c, so for a
                # fixed psum half the 16 even (or odd) rows of the group are one
                # contiguous 3584-elem run per out channel.
                for vhalf in range(2):
                    src = _v(stg[:], vhalf * OC * SGB * CFS,
                             [[SGB * CFS, OC], [1, SGB * CFS]])
                    dst = _v(out[:], (4 * SGB * og + vhalf) * W,
                             [[H * W, OC], [2 * W, 2 * SGB], [1, W]])
                    nc.sync.dma_start(out=dst, in_=src)

    return nc


def _get_nc():
    global _CACHED
    if _CACHED is None:
        nc = _build_nc()
        nc.finalize()
        _CACHED = nc
    return _CACHED


def _host_prep(input_data, foa_xy, weight):
    b = input_data.shape[0]
    wbs = np.zeros((4, 120, OC), dtype=np.float32)
    for v in range(4):
        for ci in range(C):
            for dy in range(KC):
                for dx in range(KC):
                    k = ci * 40 + dx * 8 + dy + v
                    wbs[v, k, :] = weight[:, ci, dy, dx]
    wbs = wbs.astype(ml_dtypes.bfloat16)
    idx = np.arange(H, dtype=np.float64)
    in_maps = []
    for i in range(b):
        xpad = np.zeros((C, XR, XW), dtype=ml_dtypes.bfloat16)
        xpad[:, PG:PG + H, PG:PG + W] = input_data[i].astype(ml_dtypes.bfloat16)
        fx, fy = float(foa_xy[i, 0]), float(foa_xy[i, 1])
        a_sq = (((idx - fx) / DNORM) ** 2).astype(np.float32)
        b_sq = (((idx - fy) / DNORM) ** 2).astype(np.float32)
        in_maps.append({"xp": xpad, "av": a_sq, "bv": b_sq, "wb": wbs})
    return in_maps


def kernel(input_data, foa_xy, weight):
    global LAST_RESULTS
    nc = _get_nc()
    in_maps = _host_prep(np.asarray(input_data), np.asarray(foa_xy),
                         np.asarray(weight))
    trace = bool(int(os.environ.get("BASSKERNEL_TRACE", "0")))
    res = run_bass_kernel_spmd(nc, in_maps, core_ids=list(range(8)), trace=trace)
    LAST_RESULTS = res
    outs = [np.asarray(r["out"], dtype=np.float32) for r in res.results]
    return np.stack(outs, axis=0)


# revision 15
# speedup vs baseline: 4.4707x; 1.0434x over previous
"""Trainium2 Bass kernel for FovConv2dCont (per-pixel foveated Gaussian blur + 5x5 conv).

kernel(**inputs): takes FULL inputs
  input_data f32 (8,3,224,224), foa_xy int (8,2), weight f32 (64,3,5,5)
returns f32 (8,64,224,224). Batch is data-parallel across 8 NeuronCores (1 sample/core).

Math (exact identities; bf16 storage on the heavy elementwise chain):
  gaussian tap exp(-(i^2+j^2)/(2 s^2)) = u^(i^2) * u^(j^2),  u = exp(-1/(2 s^2))
  normalizer sum over 7x7 taps = (1 + 2u + 2u^4 + 2u^9)^2
  numerator = sum over exponent classes e in {0,1,2,4,5,8,9} of u^e * S_e
  (terms e=10,13,18 dropped: bounded ~5e-3 relative, within tolerance)
  m = numerator / norm ; y = conv5x5(m, w) as K=120 matmuls with (ci,dx,dr) on the
  partition axis of an im2col buffer; 4 weight variants pre-shifted by output
  row mod 4 so the matmul partition window is always [0,120).

Pipeline per core: full-width Gaussian chain split across DVE+Pool engines ->
m field -> DRAM (mpad) -> 4 im2col DMAs -> 56 matmuls [120,128]x[120,448] ->
PSUM->SBUF copies (Scalar+Pool) -> 28 wide output DMAs (16 rows each).
"""

import os
import sys

sys.path.insert(0, "/opt/trn_rl_repo")

import numpy as np
import ml_dtypes

def _ensure_ntff_hook():
    """Register the NTFF profile hook if the image's antenv lacks axon_hooks
    (needed only for trace=True timing runs; harmless otherwise)."""
    try:
        import antenv.axon_hooks  # noqa: F401
        return
    except ImportError:
        pass
    try:
        import types
        import antenv
        import importlib.util as ilu

        spec = ilu.spec_from_file_location(
            "trn_agent_boot.trn_boot", "/root/.axon_site/trn_agent_boot/trn_boot.py"
        )
        mod = types.ModuleType("antenv.axon_hooks")
        _hook_holder = {"hook": None}

        def set_axon_ntff_profile_hook(h):
            _hook_holder["hook"] = h

        def get_axon_ntff_profile_hook():
            return _hook_holder["hook"]

        mod.set_axon_ntff_profile_hook = set_axon_ntff_profile_hook
        mod.get_axon_ntff_profile_hook = get_axon_ntff_profile_hook
        sys.modules["antenv.axon_hooks"] = mod
        antenv.axon_hooks = mod

        boot = ilu.module_from_spec(spec)
        spec.loader.exec_module(boot)
        hook = boot._ntff_profile_via_ctypes("/opt/axon/libaxon_pjrt.so")
        set_axon_ntff_profile_hook(hook)
    except Exception:
        pass


_ensure_ntff_hook()

import concourse.bass as bass
import concourse.bacc as bacc_mod
import concourse.mybir as mybir
from concourse.bass_utils import run_bass_kernel_spmd
from concourse.tile import TileContext
from concourse.alu_op_type import AluOpType

F32 = mybir.dt.float32
BF16 = mybir.dt.bfloat16
AF = mybir.ActivationFunctionType

H = W = 224
C = 3
OC = 64
KG = 7
PG = KG // 2            # 3
KC = 5
PC = KC // 2            # 2
XW = 256                # padded input row width (512B bf16 rows)
XR = H + 2 * PG         # 230 padded input rows
SR = 8                  # input rows per partition (2 out rows + gaussian halo)
NP = H // 2             # 112 partitions (row pairs)
MW = 260                # mpad row width: m cols at [2,226), im2col reads [dx, dx+256)
MR = H + 2 * PC         # 228 mpad rows
DNORM = float(np.sqrt(H * H + W * W))
NBLK = H // 4           # 56 conv row blocks
NG = 4                  # im2col block groups
GB = NBLK // NG         # 14 blocks per group
IFS = NBLK * XW         # imt free size = 56*256 = 14336
OG = 14                 # output groups (16 rows each)

# exponent classes kept (e = a^2 + b^2 over taps 0..3); 10,13,18 dropped
E_PAIR = {1: (0, 1), 4: (0, 2), 5: (1, 2), 9: (0, 3)}
E_DIAG = {2: 1, 8: 2}

LAST_RESULTS = None
_CACHED = None


def _v(ap_src, offset_elems, dims):
    """Raw strided (possibly overlapping/broadcast) view of a flat AP.
    dims = [(step, count), ...]; for SBUF/PSUM the first dim(s) must cover
    partitions (step in flat units = partition_step * free_size)."""
    fv = ap_src.flatten()
    v = fv.copy()
    v.offset = fv.offset + offset_elems
    v.ap = mybir.VecI64Pair([list(d) for d in dims])
    return v


def _build_nc():
    nc = bacc_mod.Bacc()

    xp = nc.declare_dram_parameter("xp", [C, XR, XW], BF16, isOutput=False)
    av = nc.declare_dram_parameter("av", [H], F32, isOutput=False)
    bv = nc.declare_dram_parameter("bv", [H], F32, isOutput=False)
    wb = nc.declare_dram_parameter("wb", [4, 120, OC], BF16, isOutput=False)
    out = nc.declare_dram_parameter("out", [OC, H, W], F32, isOutput=True)

    with TileContext(nc) as tc:
        with (
            tc.tile_pool(name="pers", bufs=1) as pers,
            tc.tile_pool(name="psum", bufs=8, space="PSUM") as psum_pool,
            tc.tile_pool(name="stage", bufs=3) as stage_pool,
            tc.tile_pool(name="dram", bufs=1, space="DRAM") as dram_pool,
        ):
            mpad = dram_pool.tile([C, MR, MW], BF16)

            XFS = C * SR * XW                       # xs free size 6144
            xs = pers.tile([NP, XFS], BF16)
            CFS = 2 * W                             # coeff free size 448
            at = pers.tile([NP, 2], F32)
            bvf = pers.tile([NP, W], F32)
            dist = pers.tile([NP, CFS], F32)
            sig = pers.tile([NP, CFS], F32)
            sqv = pers.tile([NP, CFS], F32)
            isg = pers.tile([NP, CFS], F32)
            u1f = pers.tile([NP, CFS], F32)
            u4f = pers.tile([NP, CFS], F32)
            u9f = pers.tile([NP, CFS], F32)
            t1 = pers.tile([NP, CFS], F32)
            t2 = pers.tile([NP, CFS], F32)
            sfield = pers.tile([NP, CFS], F32)
            rsf = pers.tile([NP, CFS], F32)
            rb = pers.tile([NP, CFS], BF16)
            ub = {e: pers.tile([NP, CFS], BF16, name=f"ub{e}")
                  for e in (1, 2, 4, 5, 8, 9)}
            RFS = C * 2 * XW                        # rowpair free size 1536
            rp = {a: pers.tile([NP, RFS], BF16, name=f"rp{a}") for a in (1, 2, 3)}
            PFS = C * 2 * W                         # P/S/T free size 1344
            pt = {k: pers.tile([NP, PFS], BF16, name=f"p{k[0]}{k[1]}")
                  for k in ((0, 1), (0, 2), (0, 3), (1, 1), (1, 2), (2, 1), (2, 2))}
            st = {e: pers.tile([NP, PFS], BF16, name=f"s{e}") for e in E_PAIR}
            tt = {e: pers.tile([NP, PFS], BF16, name=f"t{e}")
                  for e in (1, 2, 4, 5, 8, 9)}
            aa = [pers.tile([NP, PFS], BF16, name=f"aa{i}") for i in range(3)]
            ab = [pers.tile([NP, PFS], BF16, name=f"ab{i}") for i in range(2)]
            accf = pers.tile([NP, PFS], BF16)
            MFS = C * 2 * MW                        # mt free size 1560
            mt = pers.tile([NP, MFS], BF16)
            zt = pers.tile([C, 2 * 2 * MW], BF16)   # zero rows for mpad borders
            wtile = pers.tile([120, 4 * OC], BF16)
            imt = pers.tile([120, IFS], BF16)

            # ---------------- memsets (Pool engine, early) ----------------
            nc.gpsimd.memset(zt[:], 0.0)
            nc.gpsimd.memset(mt[:], 0.0)

            # ---------------- loads (sync engine; small ones first) ----------------
            nc.sync.dma_start(
                out=_v(at[:], 0, [[2, NP], [1, 2]]),
                in_=_v(av[:], 0, [[2, NP], [1, 2]]),
            )
            nc.sync.dma_start(
                out=_v(bvf[:], 0, [[W, NP], [1, W]]),
                in_=_v(bv[:], 0, [[0, NP], [1, W]]),
            )
            nc.sync.dma_start(
                out=_v(wtile[:], 0, [[4 * OC, 120], [OC, 4], [1, OC]]),
                in_=_v(wb[:], 0, [[OC, 120], [120 * OC, 4], [1, OC]]),
            )
            for ci in range(C):
                src = _v(xp[:], ci * XR * XW, [[2 * XW, NP], [XW, SR], [1, XW]])
                dst = _v(xs[:], ci * SR * XW, [[XFS, NP], [XW, SR], [1, XW]])
                nc.sync.dma_start(out=dst, in_=src)
            # mpad top/bottom zero rows (rows 0-1 and 226-227, all ci)
            nc.sync.dma_start(
                out=_v(mpad[:], 0, [[MR * MW, C], [(MR - 2) * MW, 2], [1, 2 * MW]]),
                in_=_v(zt[:], 0, [[4 * MW, C], [2 * MW, 2], [1, 2 * MW]]),
            )

            # ---------------- coefficient chain (Scalar-heavy; DVE minimal) ---
            for rh in range(2):
                nc.scalar.activation(dist[:, rh * W:(rh + 1) * W], bvf[:],
                                     AF.Sqrt, bias=at[:, rh:rh + 1])
            nc.scalar.activation(sig[:], dist[:], AF.Copy, bias=0.01, scale=0.99)
            nc.scalar.activation(sqv[:], sig[:], AF.Square)
            nc.vector.reciprocal_approx_fast(isg[:], sqv[:])
            nc.scalar.activation(u1f[:], isg[:], AF.Exp, scale=-0.5)
            nc.scalar.activation(u4f[:], isg[:], AF.Exp, scale=-2.0)
            nc.scalar.activation(u9f[:], isg[:], AF.Exp, scale=-4.5)
            for e in (1, 2, 4, 5, 8, 9):
                nc.scalar.activation(ub[e][:], isg[:], AF.Exp, scale=-0.5 * e)
            nc.vector.tensor_tensor(t1[:], u1f[:], u4f[:], AluOpType.add)
            nc.vector.tensor_tensor(t2[:], t1[:], u9f[:], AluOpType.add)
            nc.scalar.activation(sfield[:], t2[:], AF.Copy, bias=1.0, scale=2.0)
            nc.vector.reciprocal_approx_fast(rsf[:], sfield[:])
            nc.scalar.activation(rb[:], rsf[:], AF.Square)

            # ---------------- gaussian chain views ----------------
            # xs free layout: (ci, row 0..7, col 0..255); out row r of the pair
            # sits at in-partition row 3+r, center col c at 3+c.
            def xv(drow, dcol, wid):
                return _v(xs[:], (3 + drow) * XW + 3 + dcol,
                          [[XFS, NP], [SR * XW, C], [XW, 2], [1, wid]])

            def rv(t, dcol, wid):
                return _v(t[:], 3 + dcol,
                          [[RFS, NP], [2 * XW, C], [XW, 2], [1, wid]])

            def pv(t):
                return _v(t[:], 0, [[PFS, NP], [2 * W, C], [W, 2], [1, W]])

            def uv(t):
                return _v(t[:], 0, [[CFS, NP], [0, C], [W, 2], [1, W]])

            # rowpairs on DVE (full padded width)
            for a in (1, 2, 3):
                o = _v(rp[a][:], 0, [[RFS, NP], [2 * XW, C], [XW, 2], [1, XW]])
                i0 = _v(xs[:], (3 - a) * XW,
                        [[XFS, NP], [SR * XW, C], [XW, 2], [1, XW]])
                i1 = _v(xs[:], (3 + a) * XW,
                        [[XFS, NP], [SR * XW, C], [XW, 2], [1, XW]])
                nc.vector.tensor_tensor(o, i0, i1, AluOpType.add)

            # colpairs: P[a][b] = (a-rowpair or x) shifted by +-b, summed.
            # All on DVE: concurrent Pool+DVE SBUF traffic slows BOTH ~3x.
            for a, b in ((0, 1), (0, 2), (0, 3), (1, 1), (1, 2), (2, 1), (2, 2)):
                if a == 0:
                    i0, i1 = xv(0, -b, W), xv(0, +b, W)
                else:
                    i0, i1 = rv(rp[a], -b, W), rv(rp[a], +b, W)
                nc.vector.tensor_tensor(pv(pt[(a, b)]), i0, i1, AluOpType.add)

            # class sums S_e: S1=P01+rp1, S4=P02+rp2, S5=P12+P21, S9=P03+rp3
            nc.vector.tensor_tensor(pv(st[5]), pv(pt[(1, 2)]), pv(pt[(2, 1)]),
                                    AluOpType.add)
            nc.vector.tensor_tensor(pv(st[1]), pv(pt[(0, 1)]), rv(rp[1], 0, W),
                                    AluOpType.add)
            nc.vector.tensor_tensor(pv(st[4]), pv(pt[(0, 2)]), rv(rp[2], 0, W),
                                    AluOpType.add)
            nc.vector.tensor_tensor(pv(st[9]), pv(pt[(0, 3)]), rv(rp[3], 0, W),
                                    AluOpType.add)

            # products T_e = ub_e * S_e   (S2=P11, S8=P22)
            src_e = {1: st[1], 4: st[4], 5: st[5], 9: st[9],
                     2: pt[(1, 1)], 8: pt[(2, 2)]}
            for e in (1, 2, 4, 5, 8, 9):
                nc.vector.tensor_tensor(pv(tt[e]), uv(ub[e]), pv(src_e[e]),
                                        AluOpType.mult)

            # accumulation (all DVE, pairwise tree for shorter dep chains)
            nc.vector.tensor_tensor(pv(ab[0]), xv(0, 0, W), pv(tt[2]), AluOpType.add)
            nc.vector.tensor_tensor(pv(ab[1]), pv(tt[5]), pv(tt[8]), AluOpType.add)
            nc.vector.tensor_tensor(pv(aa[0]), pv(tt[1]), pv(tt[4]), AluOpType.add)
            nc.vector.tensor_tensor(pv(aa[1]), pv(tt[9]), pv(ab[0]), AluOpType.add)
            nc.vector.tensor_tensor(pv(aa[2]), pv(ab[1]), pv(aa[0]), AluOpType.add)
            nc.vector.tensor_tensor(pv(accf), pv(aa[2]), pv(aa[1]), AluOpType.add)

            # m = rb * acc, into mt center cols [2, 226)
            mdst = _v(mt[:], 2, [[MFS, NP], [2 * MW, C], [MW, 2], [1, W]])
            nc.vector.tensor_tensor(mdst, uv(rb), pv(accf), AluOpType.mult)

            # m rows -> mpad (full 260-wide rows incl zero borders)
            for ci in range(C):
                src = _v(mt[:], ci * 2 * MW, [[MFS, NP], [1, 2 * MW]])
                dst = _v(mpad[:], ci * MR * MW + 2 * MW,
                         [[2 * MW, NP], [1, 2 * MW]])
                nc.sync.dma_start(out=dst, in_=src)

            # im2col: imt[(ci*40+dx*8+dr), blk*256+j] = mpad[ci, 4blk+dr, dx+j]
            # (DMA APs are limited to 3 dims per side -> one DMA per (ci, dx);
            #  issued via gpsimd SWDGE: ~0.34ns/descriptor vs ~6ns on HWDGE)
            for ci in range(C):
                for dx in range(KC):
                    src = _v(mpad[:], ci * MR * MW + dx,
                             [[MW, SR], [4 * MW, NBLK], [1, XW]])
                    dst = _v(imt[:], (ci * 40 + dx * 8) * IFS,
                             [[IFS, SR], [XW, NBLK], [1, XW]])
                    nc.gpsimd.dma_start(out=dst, in_=src)

            # ---------------- conv: matmuls + copies + output DMA ----------------
            # group og covers out rows [32*og, 32*og+32) = blocks 8*og..8*og+7.
            # matmul (og, vp, h): lhsT = wtile[:, vp*128:(vp+1)*128] (variants
            # 2vp, 2vp+1), rhs = imt blocks (8og+2h, 8og+2h+1) -> psum [128,448];
            # psum partition p = vhalf*64+oc -> out row 4*blk+2vp+vhalf.
            SGB = 8                                 # blocks per output group
            for og in range(NBLK // SGB):
                stg = stage_pool.tile([128, SGB * CFS], F32, name="ostage")
                for vp in range(2):
                    lhsT = _v(wtile[:], vp * 128, [[4 * OC, 120], [1, 128]])
                    for h in range(SGB // 2):
                        ps = psum_pool.tile([128, CFS], F32, name="opsum")
                        rhs = _v(imt[:], (SGB * og + 2 * h) * XW,
                                 [[IFS, 120], [XW, 2], [1, W]])
                        nc.tensor.matmul(ps[:], lhsT, rhs, start=True, stop=True)
                        cdst = _v(stg[:], h * 2 * CFS + vp * W,
                                  [[SGB * CFS, 128], [CFS, 2], [1, W]])
                        csrc = _v(ps[:], 0, [[CFS, 128], [W, 2], [1, W]])
                        if vp == 0:
                            nc.scalar.copy(cdst, csrc)
                        else:
                            nc.vector.tensor_copy(cdst, csrc)
                # stage free layout is (b, vp, c) = (2b+vp)*224 + c, so for a
                # fixed psum half the 16 even (or odd) rows of the group are one
                # contiguous 3584-elem run per out channel.
                for vhalf in range(2):
                    src = _v(stg[:], vhalf * OC * SGB * CFS,
                             [[SGB * CFS, OC], [1, SGB * CFS]])
                    dst = _v(out[:], (4 * SGB * og + vhalf) * W,
                             [[H * W, OC], [2 * W, 2 * SGB], [1, W]])
                    nc.gpsimd.dma_start(out=dst, in_=src)

    return nc


def _get_nc():
    global _CACHED
    if _CACHED is None:
        nc = _build_nc()
        nc.finalize()
        _CACHED = nc
    return _CACHED


def _host_prep(input_data, foa_xy, weight):
    b = input_data.shape[0]
    wbs = np.zeros((4, 120, OC), dtype=np.float32)
    for v in range(4):
        for ci in range(C):
            for dy in range(KC):
                for dx in range(KC):
                    k = ci * 40 + dx * 8 + dy + v
                    wbs[v, k, :] = weight[:, ci, dy, dx]
    wbs = wbs.astype(ml_dtypes.bfloat16)
    idx = np.arange(H, dtype=np.float64)
    in_maps = []
    for i in range(b):
        xpad = np.zeros((C, XR, XW), dtype=ml_dtypes.bfloat16)
        xpad[:, PG:PG + H, PG:PG + W] = input_data[i].astype(ml_dtypes.bfloat16)
        fx, fy = float(foa_xy[i, 0]), float(foa_xy[i, 1])
        a_sq = (((idx - fx) / DNORM) ** 2).astype(np.float32)
        b_sq = (((idx - fy) / DNORM) ** 2).astype(np.float32)
        in_maps.append({"xp": xpad, "av": a_sq, "bv": b_sq, "wb": wbs})
    return in_maps


def kernel(input_data, foa_xy, weight):
    global LAST_RESULTS
    nc = _get_nc()
    in_maps = _host_prep(np.asarray(input_data), np.asarray(foa_xy),
                         np.asarray(weight))
    trace = bool(int(os.environ.get("BASSKERNEL_TRACE", "0")))
    res = run_bass_kernel_spmd(nc, in_maps, core_ids=list(range(8)), trace=trace)
    LAST_RESULTS = res
    outs = [np.asarray(r["out"], dtype=np.float32) for r in res.results]
    return np.stack(outs, axis=0)
